# revision 17
# baseline (speedup 1.0000x reference)
"""Trainium2 Bass kernel for nn_APIHyperInputLayer (hypernet MLP, 8-core data parallel).

Math (per branch):
    h   = prelu(F @ W1 + b1, alpha)                       [R, 64]
    w   = (h @ W2 + b2).reshape(R, F, 128)
    hid = einsum('rf,rfo->ro', F, w)
    out = hid.reshape(E, n, 128).sum(1)                   [E, 128]

Restructured: S[k,e,f] = sum_i h[(e,i),k] F[(e,i),f]; out[e,o] =
sum_{k,f} S[k,e,f] W2[k,f*128+o] + (bias term, computed on host).

v3 schedule (row-major h; no transposes):
  Episodes padded to PITCH=16 rows; 8 episodes = one 128-partition group.
  A: per group g, matmul(out=[128 rows, 128 k], lhsT=fsp[:, g*128:+128],
     rhs=w1ext) where fsp = [81, 4096] padded-transposed features with a
     ones-row; w1ext = block-diag(Wa1, We1) with bias row.  h comes out
     row-major directly; PReLU (pure, alpha=0.25) evacuates PSUM->SBUF
     once per 8 groups.
  B: per group, two 64-part matmuls sharing one PSUM tile (f-pair lo/hi in
     partition halves); rhs = block-diag masked features M built by one
     4-dim-AP DMA per branch from host-padded row-major features.
  evac: strided casts PSUM->s2[kk, fp*256+e] (vector 2/3, scalar 1/3).
  C: 40 accumulating matmuls out_T[o,e] += W2pair_fp.T @ s2 slice.
Output per core: [128 o, 256 e] fp32; host transposes/concats and adds the
bias term fsum @ b2 (host numpy, exact fp32).
"""

import os
import sys
import functools

import numpy as np

for _p in ("/opt/trn_rl_repo", os.path.expanduser("~/.axon_site/_ro/trn_rl_repo")):
    if os.path.isdir(_p) and _p not in sys.path:
        sys.path.insert(0, _p)

import dataclasses

import ml_dtypes

import concourse.bass as bass
import concourse.bacc as bacc
import concourse.mybir as mybir
import concourse.tile as tile
from concourse.bass_utils import run_bass_kernel_spmd

BF16 = mybir.dt.bfloat16
F32 = mybir.dt.float32

# Problem constants (hardcoded per contest rules)
N_CORES = 8
N_AGENTS, N_ENEMIES = 10, 11
ALLY_F, ENEMY_F = 48, 32
HYPER = 64
OUT = 128
B_FULL = 2048
E_C = B_FULL // N_CORES            # episodes per core = 256

PITCH = 16                         # padded rows per episode
EPG = 8                            # episodes per group (8*16=128 partitions)
NG = E_C // EPG                    # 32 groups
PROWS = E_C * PITCH                # padded rows per core = 4096
GW_A = EPG * ALLY_F                # 384 M_a cols per group
GW_E = EPG * ENEMY_F               # 256
PAIR_A = ALLY_F // 2               # 24
PAIR_E = ENEMY_F // 2              # 16
MA_FREE = NG * GW_A                # 12288
ME_FREE = NG * GW_E                # 8192
S2A_FREE = PAIR_A * E_C            # 6144
S2E_FREE = PAIR_E * E_C            # 4096
W2COLS = (PAIR_A + PAIR_E) * OUT   # 5120
KROWS = ALLY_F + ENEMY_F + 1       # 81 = stacked features + ones row

BATCH = 8                          # groups per prelu batch (8*128 = 1024 cols)
NBATCH = NG // BATCH               # 4
IMG_G = 8                          # leading groups delivered as dense host image
IMA_COLS = IMG_G * GW_A            # 3072
IME_COLS = IMG_G * GW_E            # 2048


def _ap(t, offset, dims):
    """Custom flat AP: dims = [(step, num), ...]; t is an AP or tensor handle."""
    a = t if isinstance(t, bass.AP) else t.ap()
    return dataclasses.replace(a, offset=offset, ap=[[s, n] for (s, n) in dims])


def build_program(alpha_a=0.25, alpha_e=0.25):
    assert alpha_a == alpha_e, "branches must share alpha"
    nc = bacc.Bacc("TRN2", target_bir_lowering=False, debug=False)

    fsp = nc.declare_dram_parameter("fsp", [KROWS, PROWS], BF16, isOutput=False)
    ma_img = nc.declare_dram_parameter("ma_img", [128, MA_FREE], BF16, isOutput=False)
    me_img = nc.declare_dram_parameter("me_img", [128, ME_FREE], BF16, isOutput=False)
    w1e = nc.declare_dram_parameter("w1e", [KROWS, 128], BF16, isOutput=False)
    w2pack = nc.declare_dram_parameter("w2pack", [128, W2COLS], BF16, isOutput=False)
    out_d = nc.declare_dram_parameter("out", [OUT, E_C], F32, isOutput=True)

    with tile.TileContext(nc) as tc:
        _emit(nc, tc, fsp, ma_img, me_img, w1e, w2pack, out_d, alpha_a)
    nc.compile()
    return nc


def _emit(nc, tc, fsp, ma_img, me_img, w1e, w2pack, out_d, alpha):
    from contextlib import ExitStack

    Prelu = mybir.ActivationFunctionType.Prelu

    ctx = ExitStack()
    with ctx:
        const = ctx.enter_context(tc.tile_pool(name="const", bufs=1))
        work = ctx.enter_context(tc.tile_pool(name="work", bufs=1))
        psA = ctx.enter_context(tc.tile_pool(name="psA", bufs=3, space="PSUM"))
        psB = ctx.enter_context(tc.tile_pool(name="psB", bufs=3, space="PSUM"))
        psC = ctx.enter_context(tc.tile_pool(name="psC", bufs=1, space="PSUM"))
        psW = ctx.enter_context(tc.tile_pool(name="psW", bufs=1, space="PSUM"))

        # ---- persistent SBUF ----
        fsp_sb = const.tile([KROWS, PROWS], BF16)
        w1_sb = const.tile([KROWS, 128], BF16)
        w2_sb = const.tile([128, W2COLS], BF16)
        h_sb = work.tile([128, PROWS], BF16)
        ma_sb = work.tile([128, MA_FREE], BF16)
        me_sb = work.tile([128, ME_FREE], BF16)
        s2a = work.tile([128, S2A_FREE], BF16)
        s2e = work.tile([128, S2E_FREE], BF16)
        osb = work.tile([OUT, E_C], F32)

        # ---- PE warmup: junk matmuls to flip the HAM clock gate to 8/8
        # before real work arrives (reads a memset junk tile) ----
        junk = work.tile([128, 64], BF16)
        nc.vector.memset(junk[:].bitcast(F32), 0.0)
        pw = psW.tile([128, 512], F32)
        for wi in range(56):
            nc.tensor.matmul(pw[0:64, 0:64], junk[:], junk[:],
                             start=True, stop=True)

        # ---- parameter loads: all clean dense line-rate DMAs, no memsets ----
        # sync ring: fsp chunk 0, w2a, M-ally image chunks (one per batch).
        # scalar ring: w1, fsp chunk 1, w2b, M-enemy image chunks.
        CHW = PROWS // 2
        nc.sync.dma_start(fsp_sb[:, 0:CHW], fsp.ap()[:, 0:CHW])
        nc.scalar.dma_start(w1_sb[:], w1e.ap())
        nc.scalar.dma_start(
            fsp_sb[:, CHW : 2 * CHW], fsp.ap()[:, CHW : 2 * CHW])
        nc.sync.dma_start(w2_sb[:, 0 : W2COLS // 2], w2pack.ap()[:, 0 : W2COLS // 2])
        nc.scalar.dma_start(w2_sb[:, W2COLS // 2 :], w2pack.ap()[:, W2COLS // 2 :])
        MAC = MA_FREE // NBATCH     # ally image cols per batch chunk
        MEC = ME_FREE // NBATCH
        for b in range(NBATCH):
            nc.sync.dma_start(
                ma_sb[:, b * MAC : (b + 1) * MAC],
                ma_img.ap()[:, b * MAC : (b + 1) * MAC])
            nc.gpsimd.dma_start(
                me_sb[:, b * MEC : (b + 1) * MEC],
                me_img.ap()[:, b * MEC : (b + 1) * MEC])

        # ---- pipeline: per sub-batch of 4 groups: A matmuls + prelu;
        # per batch of 8 groups: B matmuls + evac casts ----
        copy_rr = [0]

        def evac(dst, src):
            if copy_rr[0] % 3 == 2:
                nc.scalar.copy(dst, src)
            else:
                nc.vector.tensor_copy(dst, src)
            copy_rr[0] += 1

        SUBG = 4                     # groups per psA tile / prelu

        def a_stage(sub):
            pa = psA.tile([128, SUBG * 128], F32, tag="psA")
            for j in range(SUBG):
                g = sub * SUBG + j
                nc.tensor.matmul(
                    pa[:, j * 128 : (j + 1) * 128],
                    fsp_sb[:, g * 128 : (g + 1) * 128],
                    w1_sb[:],
                    start=True, stop=True)
            nc.scalar.activation(
                h_sb[:, sub * 512 : (sub + 1) * 512], pa[:],
                Prelu, scale=1.0, alpha=alpha)

        def b_ally(g0):              # 2 groups per PSUM tile
            pb = psB.tile([128, 512], F32, tag="psB")
            for u in range(2):
                g = g0 + u
                lhsT = h_sb[:, g * 128 : g * 128 + 64]
                nc.tensor.matmul(
                    pb[64:128, u * 192 : (u + 1) * 192], lhsT,
                    _ap(ma_sb, g * GW_A + PAIR_A,
                        [(MA_FREE, 128), (ALLY_F, EPG), (1, PAIR_A)]),
                    start=True, stop=True)
                nc.tensor.matmul(
                    pb[0:64, u * 192 : (u + 1) * 192], lhsT,
                    _ap(ma_sb, g * GW_A,
                        [(MA_FREE, 128), (ALLY_F, EPG), (1, PAIR_A)]),
                    start=True, stop=True)
            evac(
                _ap(s2a, g0 * EPG,
                    [(S2A_FREE, 128), (E_C, PAIR_A), (1, 2 * EPG)]),
                _ap(pb, 0,
                    [(512, 128), (1, PAIR_A), (192, 2), (PAIR_A, EPG)]))

        def b_enemy(g0):             # 4 groups per PSUM tile
            pb = psB.tile([128, 512], F32, tag="psB")
            for u in range(4):
                g = g0 + u
                lhsT = h_sb[:, g * 128 + 64 : g * 128 + 128]
                nc.tensor.matmul(
                    pb[64:128, u * 128 : (u + 1) * 128], lhsT,
                    _ap(me_sb, g * GW_E + PAIR_E,
                        [(ME_FREE, 128), (ENEMY_F, EPG), (1, PAIR_E)]),
                    start=True, stop=True)
                nc.tensor.matmul(
                    pb[0:64, u * 128 : (u + 1) * 128], lhsT,
                    _ap(me_sb, g * GW_E,
                        [(ME_FREE, 128), (ENEMY_F, EPG), (1, PAIR_E)]),
                    start=True, stop=True)
            evac(
                _ap(s2e, g0 * EPG,
                    [(S2E_FREE, 128), (E_C, PAIR_E), (1, 4 * EPG)]),
                _ap(pb, 0,
                    [(512, 128), (1, PAIR_E), (128, 4), (PAIR_E, EPG)]))

        for sub in range(NG // SUBG):
            a_stage(sub)
        for g0 in range(0, NG, 2):
            b_ally(g0)
        for g0 in range(0, NG, 4):
            b_enemy(g0)

        # ---- stage C: out_T[o,e] accumulation over 40 fp slices ----
        pc = psC.tile([OUT, E_C], F32)
        n_sl = PAIR_A + PAIR_E
        idx = 0
        for fp in range(PAIR_A):
            nc.tensor.matmul(
                pc[:], w2_sb[:, fp * OUT : (fp + 1) * OUT],
                s2a[:, fp * E_C : (fp + 1) * E_C],
                start=(idx == 0), stop=(idx == n_sl - 1))
            idx += 1
        for fp in range(PAIR_E):
            nc.tensor.matmul(
                pc[:], w2_sb[:, (PAIR_A + fp) * OUT : (PAIR_A + fp + 1) * OUT],
                s2e[:, fp * E_C : (fp + 1) * E_C],
                start=(idx == 0), stop=(idx == n_sl - 1))
            idx += 1

        nc.vector.tensor_copy(osb[:], pc[:])
        nc.sync.dma_start(out_d.ap(), osb[:])


@functools.lru_cache(maxsize=2)
def _cached_program(alpha_a, alpha_e):
    return build_program(alpha_a, alpha_e)


def host_prep(ally_features, enemy_features, Wa1, ba1, aa, Wa2, ba2,
              We1, be1, ae, We2, be2):
    """Per-core input maps (numpy, bf16) + host-side bias term."""
    bf = ml_dtypes.bfloat16

    def uniform_alpha(a):
        a = np.asarray(a, dtype=np.float32)
        assert np.allclose(a, a[0]), "per-channel alpha not supported"
        return float(a[0])

    ua, ue = uniform_alpha(aa), uniform_alpha(ae)

    w1 = np.zeros((KROWS, 128), dtype=np.float32)
    w1[0:ALLY_F, 0:HYPER] = np.asarray(Wa1)
    w1[ALLY_F:80, HYPER:128] = np.asarray(We1)
    w1[80, 0:HYPER] = np.asarray(ba1)
    w1[80, HYPER:128] = np.asarray(be1)
    w1 = w1.astype(bf)

    w2 = np.zeros((128, W2COLS), dtype=np.float32)
    Wa2_, We2_ = np.asarray(Wa2), np.asarray(We2)
    for fp in range(PAIR_A):
        w2[0:HYPER, fp * OUT : (fp + 1) * OUT] = Wa2_[:, fp * OUT : (fp + 1) * OUT]
        w2[HYPER:128, fp * OUT : (fp + 1) * OUT] = \
            Wa2_[:, (fp + PAIR_A) * OUT : (fp + PAIR_A + 1) * OUT]
    for fp in range(PAIR_E):
        c = (PAIR_A + fp) * OUT
        w2[0:HYPER, c : c + OUT] = We2_[:, fp * OUT : (fp + 1) * OUT]
        w2[HYPER:128, c : c + OUT] = \
            We2_[:, (fp + PAIR_E) * OUT : (fp + PAIR_E + 1) * OUT]
    w2 = w2.astype(bf)

    fa_all = np.asarray(ally_features, dtype=np.float32)
    fe_all = np.asarray(enemy_features, dtype=np.float32)
    fa_bf = fa_all.astype(bf)
    fe_bf = fe_all.astype(bf)

    # host-side bias term: fsum @ b2 (exact fp32)
    fsum_a = fa_all.reshape(B_FULL, N_AGENTS, ALLY_F).sum(axis=1)
    fsum_e = fe_all.reshape(B_FULL, N_ENEMIES, ENEMY_F).sum(axis=1)
    bias_out = (fsum_a @ np.asarray(ba2).reshape(ALLY_F, OUT)
                + fsum_e @ np.asarray(be2).reshape(ENEMY_F, OUT)).astype(np.float32)

    RA = E_C * N_AGENTS
    RE = E_C * N_ENEMIES
    in_maps = []
    for c in range(N_CORES):
        fa_c = fa_bf[c * RA : (c + 1) * RA].reshape(E_C, N_AGENTS, ALLY_F)
        fe_c = fe_bf[c * RE : (c + 1) * RE].reshape(E_C, N_ENEMIES, ENEMY_F)
        # padded row-major features [E_C*16, featf], pad rows zero
        fa_pad = np.zeros((E_C, PITCH, ALLY_F), dtype=bf)
        fa_pad[:, :N_AGENTS, :] = fa_c
        fe_pad = np.zeros((E_C, PITCH, ENEMY_F), dtype=bf)
        fe_pad[:, :N_ENEMIES, :] = fe_c
        fa_pad = fa_pad.reshape(PROWS, ALLY_F)
        fe_pad = fe_pad.reshape(PROWS, ENEMY_F)
        # fsp: [81, PROWS] = [ally F^T padded; enemy F^T padded; ones]
        fs = np.zeros((KROWS, PROWS), dtype=bf)
        fs[0:ALLY_F, :] = fa_pad.T
        fs[ALLY_F:80, :] = fe_pad.T
        fs[80, :] = np.float32(1.0)

        # dense block-diag M images: [p=(el,i), col=g*gw + el*featf + f]
        def m_image(f_pad, featf, cols):
            img = np.zeros((EPG, PITCH, NG, EPG, featf), dtype=bf)
            f4 = f_pad.reshape(NG, EPG, PITCH, featf)
            for el in range(EPG):
                img[el, :, :, el, :] = f4[:, el].transpose(1, 0, 2)
            return np.ascontiguousarray(img.reshape(128, cols))

        in_maps.append({
            "fsp": np.ascontiguousarray(fs),
            "ma_img": m_image(fa_pad, ALLY_F, MA_FREE),
            "me_img": m_image(fe_pad, ENEMY_F, ME_FREE),
            "w1e": w1, "w2pack": w2,
        })
    aux = {"bias_out": bias_out, "ua": ua, "ue": ue}
    return in_maps, aux


def assemble_output(results, aux):
    outs = [np.asarray(r["out"], dtype=np.float32) for r in results]
    dev = np.concatenate([o.T for o in outs], axis=0)
    return dev + aux["bias_out"]


def kernel(**inputs) -> np.ndarray:
    in_maps, aux = host_prep(**inputs)
    nc = _cached_program(aux["ua"], aux["ue"])
    res = run_bass_kernel_spmd(nc, in_maps, core_ids=list(range(N_CORES)))
    return assemble_output(res.results, aux)


if __name__ == "__main__":
    build_program()
    print("built ok")


# revision 18
# speedup vs baseline: 1.1552x; 1.1552x over previous
"""Trainium2 Bass kernel for nn_APIHyperInputLayer (hypernet MLP, 8-core data parallel).

Math (per branch):
    h   = prelu(F @ W1 + b1, alpha)                       [R, 64]
    w   = (h @ W2 + b2).reshape(R, F, 128)
    hid = einsum('rf,rfo->ro', F, w)
    out = hid.reshape(E, n, 128).sum(1)                   [E, 128]

Restructured: S[k,e,f] = sum_i h[(e,i),k] F[(e,i),f]; out[e,o] =
sum_{k,f} S[k,e,f] W2[k,f*128+o] + (bias term, computed on host).

v3 schedule (row-major h; no transposes):
  Episodes padded to PITCH=16 rows; 8 episodes = one 128-partition group.
  A: per group g, matmul(out=[128 rows, 128 k], lhsT=fsp[:, g*128:+128],
     rhs=w1ext) where fsp = [81, 4096] padded-transposed features with a
     ones-row; w1ext = block-diag(Wa1, We1) with bias row.  h comes out
     row-major directly; PReLU (pure, alpha=0.25) evacuates PSUM->SBUF
     once per 8 groups.
  B: per group, two 64-part matmuls sharing one PSUM tile (f-pair lo/hi in
     partition halves); rhs = block-diag masked features M built by one
     4-dim-AP DMA per branch from host-padded row-major features.
  evac: strided casts PSUM->s2[kk, fp*256+e] (vector 2/3, scalar 1/3).
  C: 40 accumulating matmuls out_T[o,e] += W2pair_fp.T @ s2 slice.
Output per core: [128 o, 256 e] fp32; host transposes/concats and adds the
bias term fsum @ b2 (host numpy, exact fp32).
"""

import os
import sys
import functools

import numpy as np

for _p in ("/opt/trn_rl_repo", os.path.expanduser("~/.axon_site/_ro/trn_rl_repo")):
    if os.path.isdir(_p) and _p not in sys.path:
        sys.path.insert(0, _p)

import dataclasses

import ml_dtypes

import concourse.bass as bass
import concourse.bacc as bacc
import concourse.mybir as mybir
import concourse.tile as tile
from concourse.bass_utils import run_bass_kernel_spmd

BF16 = mybir.dt.bfloat16
F32 = mybir.dt.float32

# Problem constants (hardcoded per contest rules)
N_CORES = 8
N_AGENTS, N_ENEMIES = 10, 11
ALLY_F, ENEMY_F = 48, 32
HYPER = 64
OUT = 128
B_FULL = 2048
E_C = B_FULL // N_CORES            # episodes per core = 256

PITCH = 16                         # padded rows per episode
EPG = 8                            # episodes per group (8*16=128 partitions)
NG = E_C // EPG                    # 32 groups
PROWS = E_C * PITCH                # padded rows per core = 4096
GW_A = EPG * ALLY_F                # 384 M_a cols per group
GW_E = EPG * ENEMY_F               # 256
PAIR_A = ALLY_F // 2               # 24
PAIR_E = ENEMY_F // 2              # 16
MA_FREE = NG * GW_A                # 12288
ME_FREE = NG * GW_E                # 8192
S2A_FREE = PAIR_A * E_C            # 6144
S2E_FREE = PAIR_E * E_C            # 4096
W2COLS = (PAIR_A + PAIR_E) * OUT   # 5120
KROWS = ALLY_F + ENEMY_F + 1       # 81 = stacked features + ones row

BATCH = 8                          # groups per prelu batch (8*128 = 1024 cols)
NBATCH = NG // BATCH               # 4
IMG_G = 8                          # leading groups delivered as dense host image
IMA_COLS = IMG_G * GW_A            # 3072
IME_COLS = IMG_G * GW_E            # 2048


def _ap(t, offset, dims):
    """Custom flat AP: dims = [(step, num), ...]; t is an AP or tensor handle."""
    a = t if isinstance(t, bass.AP) else t.ap()
    return dataclasses.replace(a, offset=offset, ap=[[s, n] for (s, n) in dims])


def build_program(alpha_a=0.25, alpha_e=0.25):
    assert alpha_a == alpha_e, "branches must share alpha"
    nc = bacc.Bacc("TRN2", target_bir_lowering=False, debug=False)

    fsp = nc.declare_dram_parameter("fsp", [KROWS, PROWS], BF16, isOutput=False)
    fapx = nc.declare_dram_parameter("fapx", [128, NG * ALLY_F], BF16, isOutput=False)
    fepx = nc.declare_dram_parameter("fepx", [128, NG * ENEMY_F], BF16, isOutput=False)
    w1e = nc.declare_dram_parameter("w1e", [KROWS, 128], BF16, isOutput=False)
    w2pack = nc.declare_dram_parameter("w2pack", [128, W2COLS], BF16, isOutput=False)
    out_d = nc.declare_dram_parameter("out", [OUT, E_C], F32, isOutput=True)

    with tile.TileContext(nc) as tc:
        _emit(nc, tc, fsp, fapx, fepx, w1e, w2pack, out_d, alpha_a)
    nc.compile()
    return nc


def _emit(nc, tc, fsp, fapx, fepx, w1e, w2pack, out_d, alpha):
    from contextlib import ExitStack

    Prelu = mybir.ActivationFunctionType.Prelu

    ctx = ExitStack()
    with ctx:
        const = ctx.enter_context(tc.tile_pool(name="const", bufs=1))
        work = ctx.enter_context(tc.tile_pool(name="work", bufs=1))
        psA = ctx.enter_context(tc.tile_pool(name="psA", bufs=3, space="PSUM"))
        psB = ctx.enter_context(tc.tile_pool(name="psB", bufs=3, space="PSUM"))
        psC = ctx.enter_context(tc.tile_pool(name="psC", bufs=1, space="PSUM"))
        psW = ctx.enter_context(tc.tile_pool(name="psW", bufs=1, space="PSUM"))

        # ---- persistent SBUF ----
        fsp_sb = const.tile([KROWS, PROWS], BF16)
        w1_sb = const.tile([KROWS, 128], BF16)
        w2_sb = const.tile([128, W2COLS], BF16)
        h_sb = work.tile([128, PROWS], BF16)
        ma_sb = work.tile([128, MA_FREE], BF16)
        me_sb = work.tile([128, ME_FREE], BF16)
        s2a = work.tile([128, S2A_FREE], BF16)
        s2e = work.tile([128, S2E_FREE], BF16)
        osb = work.tile([OUT, E_C], F32)

        # ---- PE warmup: junk matmuls to flip the HAM clock gate to 8/8
        # before real work arrives (reads a memset junk tile) ----
        junk = work.tile([128, 64], BF16)
        nc.vector.memset(junk[:].bitcast(F32), 0.0)
        pw = psW.tile([128, 512], F32)
        for wi in range(56):
            nc.tensor.matmul(pw[0:64, 0:64], junk[:], junk[:],
                             start=True, stop=True)

        # ---- memsets for M off-diagonal zeros (vector + gpsimd) ----
        ma_f32 = ma_sb[:].bitcast(F32)
        me_f32 = me_sb[:].bitcast(F32)
        nc.vector.memset(ma_f32[:, 0 : MA_FREE // 4], 0.0)
        nc.gpsimd.memset(ma_f32[:, MA_FREE // 4 : MA_FREE // 2], 0.0)
        nc.vector.memset(me_f32[:, 0 : ME_FREE // 4], 0.0)
        nc.gpsimd.memset(me_f32[:, ME_FREE // 4 : ME_FREE // 2], 0.0)

        # ---- parameter loads ----
        CHW = PROWS // 2
        nc.sync.dma_start(fsp_sb[:, 0:CHW], fsp.ap()[:, 0:CHW])
        nc.scalar.dma_start(w1_sb[:], w1e.ap())
        nc.scalar.dma_start(
            fsp_sb[:, CHW : 2 * CHW], fsp.ap()[:, CHW : 2 * CHW])
        nc.sync.dma_start(w2_sb[:, 0 : W2COLS // 2], w2pack.ap()[:, 0 : W2COLS // 2])
        nc.scalar.dma_start(w2_sb[:, W2COLS // 2 :], w2pack.ap()[:, W2COLS // 2 :])

        # M diag el-DMAs (el-major layout: col = el*gwT + g*featf + f).
        # Each writes [16 partitions x gwT contiguous cols] -- clean
        # line-rate descriptors from the compact host array.
        def el_dma(eng, el, m_sb, f_d, featf, mfree):
            gwT = NG * featf
            eng.dma_start(
                _ap(m_sb, el * (PITCH * mfree + gwT), [
                    (mfree, PITCH), (1, gwT)]),
                f_d.ap()[el * PITCH : (el + 1) * PITCH, :])

        ELQ = {0: nc.sync, 1: nc.sync, 2: nc.sync, 3: nc.sync,
               4: nc.scalar, 5: nc.scalar, 6: nc.gpsimd, 7: nc.gpsimd}
        for el in range(EPG):
            el_dma(ELQ[el], el, ma_sb, fapx, ALLY_F, MA_FREE)
        EEQ = {0: nc.sync, 1: nc.sync, 2: nc.gpsimd, 3: nc.gpsimd,
               4: nc.gpsimd, 5: nc.gpsimd, 6: nc.sync, 7: nc.sync}
        for el in range(EPG):
            el_dma(EEQ[el], el, me_sb, fepx, ENEMY_F, ME_FREE)

        # ---- pipeline: per sub-batch of 4 groups: A matmuls + prelu;
        # per batch of 8 groups: B matmuls + evac casts ----
        copy_rr = [0]

        def evac(dst, src):
            if copy_rr[0] % 3 == 2:
                nc.scalar.copy(dst, src)
            else:
                nc.vector.tensor_copy(dst, src)
            copy_rr[0] += 1

        SUBG = 4                     # groups per psA tile / prelu

        def a_stage(sub):
            pa = psA.tile([128, SUBG * 128], F32, tag="psA")
            for j in range(SUBG):
                g = sub * SUBG + j
                nc.tensor.matmul(
                    pa[:, j * 128 : (j + 1) * 128],
                    fsp_sb[:, g * 128 : (g + 1) * 128],
                    w1_sb[:],
                    start=True, stop=True)
            nc.scalar.activation(
                h_sb[:, sub * 512 : (sub + 1) * 512], pa[:],
                Prelu, scale=1.0, alpha=alpha)

        def b_ally(g0):              # 2 groups per PSUM tile
            pb = psB.tile([128, 512], F32, tag="psB")
            for u in range(2):
                g = g0 + u
                lhsT = h_sb[:, g * 128 : g * 128 + 64]
                nc.tensor.matmul(
                    pb[64:128, u * 192 : (u + 1) * 192], lhsT,
                    _ap(ma_sb, g * ALLY_F + PAIR_A,
                        [(MA_FREE, 128), (NG * ALLY_F, EPG), (1, PAIR_A)]),
                    start=True, stop=True)
                nc.tensor.matmul(
                    pb[0:64, u * 192 : (u + 1) * 192], lhsT,
                    _ap(ma_sb, g * ALLY_F,
                        [(MA_FREE, 128), (NG * ALLY_F, EPG), (1, PAIR_A)]),
                    start=True, stop=True)
            evac(
                _ap(s2a, g0 * EPG,
                    [(S2A_FREE, 128), (E_C, PAIR_A), (1, 2 * EPG)]),
                _ap(pb, 0,
                    [(512, 128), (1, PAIR_A), (192, 2), (PAIR_A, EPG)]))

        def b_enemy(g0):             # 4 groups per PSUM tile
            pb = psB.tile([128, 512], F32, tag="psB")
            for u in range(4):
                g = g0 + u
                lhsT = h_sb[:, g * 128 + 64 : g * 128 + 128]
                nc.tensor.matmul(
                    pb[64:128, u * 128 : (u + 1) * 128], lhsT,
                    _ap(me_sb, g * ENEMY_F + PAIR_E,
                        [(ME_FREE, 128), (NG * ENEMY_F, EPG), (1, PAIR_E)]),
                    start=True, stop=True)
                nc.tensor.matmul(
                    pb[0:64, u * 128 : (u + 1) * 128], lhsT,
                    _ap(me_sb, g * ENEMY_F,
                        [(ME_FREE, 128), (NG * ENEMY_F, EPG), (1, PAIR_E)]),
                    start=True, stop=True)
            evac(
                _ap(s2e, g0 * EPG,
                    [(S2E_FREE, 128), (E_C, PAIR_E), (1, 4 * EPG)]),
                _ap(pb, 0,
                    [(512, 128), (1, PAIR_E), (128, 4), (PAIR_E, EPG)]))

        for sub in range(NG // SUBG):
            a_stage(sub)
        for g0 in range(0, NG, 2):
            b_ally(g0)
        for g0 in range(0, NG, 4):
            b_enemy(g0)

        # ---- stage C: out_T[o,e] accumulation over 40 fp slices ----
        pc = psC.tile([OUT, E_C], F32)
        n_sl = PAIR_A + PAIR_E
        idx = 0
        for fp in range(PAIR_A):
            nc.tensor.matmul(
                pc[:], w2_sb[:, fp * OUT : (fp + 1) * OUT],
                s2a[:, fp * E_C : (fp + 1) * E_C],
                start=(idx == 0), stop=(idx == n_sl - 1))
            idx += 1
        for fp in range(PAIR_E):
            nc.tensor.matmul(
                pc[:], w2_sb[:, (PAIR_A + fp) * OUT : (PAIR_A + fp + 1) * OUT],
                s2e[:, fp * E_C : (fp + 1) * E_C],
                start=(idx == 0), stop=(idx == n_sl - 1))
            idx += 1

        nc.vector.tensor_copy(osb[:], pc[:])
        nc.sync.dma_start(out_d.ap(), osb[:])


@functools.lru_cache(maxsize=2)
def _cached_program(alpha_a, alpha_e):
    return build_program(alpha_a, alpha_e)


def host_prep(ally_features, enemy_features, Wa1, ba1, aa, Wa2, ba2,
              We1, be1, ae, We2, be2):
    """Per-core input maps (numpy, bf16) + host-side bias term."""
    bf = ml_dtypes.bfloat16

    def uniform_alpha(a):
        a = np.asarray(a, dtype=np.float32)
        assert np.allclose(a, a[0]), "per-channel alpha not supported"
        return float(a[0])

    ua, ue = uniform_alpha(aa), uniform_alpha(ae)

    w1 = np.zeros((KROWS, 128), dtype=np.float32)
    w1[0:ALLY_F, 0:HYPER] = np.asarray(Wa1)
    w1[ALLY_F:80, HYPER:128] = np.asarray(We1)
    w1[80, 0:HYPER] = np.asarray(ba1)
    w1[80, HYPER:128] = np.asarray(be1)
    w1 = w1.astype(bf)

    w2 = np.zeros((128, W2COLS), dtype=np.float32)
    Wa2_, We2_ = np.asarray(Wa2), np.asarray(We2)
    for fp in range(PAIR_A):
        w2[0:HYPER, fp * OUT : (fp + 1) * OUT] = Wa2_[:, fp * OUT : (fp + 1) * OUT]
        w2[HYPER:128, fp * OUT : (fp + 1) * OUT] = \
            Wa2_[:, (fp + PAIR_A) * OUT : (fp + PAIR_A + 1) * OUT]
    for fp in range(PAIR_E):
        c = (PAIR_A + fp) * OUT
        w2[0:HYPER, c : c + OUT] = We2_[:, fp * OUT : (fp + 1) * OUT]
        w2[HYPER:128, c : c + OUT] = \
            We2_[:, (fp + PAIR_E) * OUT : (fp + PAIR_E + 1) * OUT]
    w2 = w2.astype(bf)

    fa_all = np.asarray(ally_features, dtype=np.float32)
    fe_all = np.asarray(enemy_features, dtype=np.float32)
    fa_bf = fa_all.astype(bf)
    fe_bf = fe_all.astype(bf)

    # host-side bias term: fsum @ b2 (exact fp32)
    fsum_a = fa_all.reshape(B_FULL, N_AGENTS, ALLY_F).sum(axis=1)
    fsum_e = fe_all.reshape(B_FULL, N_ENEMIES, ENEMY_F).sum(axis=1)
    bias_out = (fsum_a @ np.asarray(ba2).reshape(ALLY_F, OUT)
                + fsum_e @ np.asarray(be2).reshape(ENEMY_F, OUT)).astype(np.float32)

    RA = E_C * N_AGENTS
    RE = E_C * N_ENEMIES
    in_maps = []
    for c in range(N_CORES):
        fa_c = fa_bf[c * RA : (c + 1) * RA].reshape(E_C, N_AGENTS, ALLY_F)
        fe_c = fe_bf[c * RE : (c + 1) * RE].reshape(E_C, N_ENEMIES, ENEMY_F)
        # padded row-major features [E_C*16, featf], pad rows zero
        fa_pad = np.zeros((E_C, PITCH, ALLY_F), dtype=bf)
        fa_pad[:, :N_AGENTS, :] = fa_c
        fe_pad = np.zeros((E_C, PITCH, ENEMY_F), dtype=bf)
        fe_pad[:, :N_ENEMIES, :] = fe_c
        fa_pad = fa_pad.reshape(PROWS, ALLY_F)
        fe_pad = fe_pad.reshape(PROWS, ENEMY_F)
        # fsp: [81, PROWS] = [ally F^T padded; enemy F^T padded; ones]
        fs = np.zeros((KROWS, PROWS), dtype=bf)
        fs[0:ALLY_F, :] = fa_pad.T
        fs[ALLY_F:80, :] = fe_pad.T
        fs[80, :] = np.float32(1.0)

        # compact el-major diag arrays: row p=(el*16+i), col g*featf + f
        def m_compact(f_pad, featf):
            f4 = f_pad.reshape(NG, EPG, PITCH, featf)
            return np.ascontiguousarray(
                f4.transpose(1, 2, 0, 3).reshape(128, NG * featf))

        in_maps.append({
            "fsp": np.ascontiguousarray(fs),
            "fapx": m_compact(fa_pad, ALLY_F),
            "fepx": m_compact(fe_pad, ENEMY_F),
            "w1e": w1, "w2pack": w2,
        })
    aux = {"bias_out": bias_out, "ua": ua, "ue": ue}
    return in_maps, aux


def assemble_output(results, aux):
    outs = [np.asarray(r["out"], dtype=np.float32) for r in results]
    dev = np.concatenate([o.T for o in outs], axis=0)
    return dev + aux["bias_out"]


def kernel(**inputs) -> np.ndarray:
    in_maps, aux = host_prep(**inputs)
    nc = _cached_program(aux["ua"], aux["ue"])
    res = run_bass_kernel_spmd(nc, in_maps, core_ids=list(range(N_CORES)))
    return assemble_output(res.results, aux)


if __name__ == "__main__":
    build_program()
    print("built ok")


# revision 19
# speedup vs baseline: 1.1595x; 1.0037x over previous
"""Trainium2 Bass kernel for nn_APIHyperInputLayer (hypernet MLP, 8-core data parallel).

Math (per branch):
    h   = prelu(F @ W1 + b1, alpha)                       [R, 64]
    w   = (h @ W2 + b2).reshape(R, F, 128)
    hid = einsum('rf,rfo->ro', F, w)
    out = hid.reshape(E, n, 128).sum(1)                   [E, 128]

Restructured: S[k,e,f] = sum_i h[(e,i),k] F[(e,i),f]; out[e,o] =
sum_{k,f} S[k,e,f] W2[k,f*128+o] + (bias term, computed on host).

v3 schedule (row-major h; no transposes):
  Episodes padded to PITCH=16 rows; 8 episodes = one 128-partition group.
  A: per group g, matmul(out=[128 rows, 128 k], lhsT=fsp[:, g*128:+128],
     rhs=w1ext) where fsp = [81, 4096] padded-transposed features with a
     ones-row; w1ext = block-diag(Wa1, We1) with bias row.  h comes out
     row-major directly; PReLU (pure, alpha=0.25) evacuates PSUM->SBUF
     once per 8 groups.
  B: per group, two 64-part matmuls sharing one PSUM tile (f-pair lo/hi in
     partition halves); rhs = block-diag masked features M built by one
     4-dim-AP DMA per branch from host-padded row-major features.
  evac: strided casts PSUM->s2[kk, fp*256+e] (vector 2/3, scalar 1/3).
  C: 40 accumulating matmuls out_T[o,e] += W2pair_fp.T @ s2 slice.
Output per core: [128 o, 256 e] fp32; host transposes/concats and adds the
bias term fsum @ b2 (host numpy, exact fp32).
"""

import os
import sys
import functools

import numpy as np

for _p in ("/opt/trn_rl_repo", os.path.expanduser("~/.axon_site/_ro/trn_rl_repo")):
    if os.path.isdir(_p) and _p not in sys.path:
        sys.path.insert(0, _p)

import dataclasses

import ml_dtypes

import concourse.bass as bass
import concourse.bacc as bacc
import concourse.mybir as mybir
import concourse.tile as tile
from concourse.bass_utils import run_bass_kernel_spmd

BF16 = mybir.dt.bfloat16
F32 = mybir.dt.float32

# Problem constants (hardcoded per contest rules)
N_CORES = 8
N_AGENTS, N_ENEMIES = 10, 11
ALLY_F, ENEMY_F = 48, 32
HYPER = 64
OUT = 128
B_FULL = 2048
E_C = B_FULL // N_CORES            # episodes per core = 256

PITCH = 16                         # padded rows per episode
EPG = 8                            # episodes per group (8*16=128 partitions)
NG = E_C // EPG                    # 32 groups
PROWS = E_C * PITCH                # padded rows per core = 4096
GW_A = EPG * ALLY_F                # 384 M_a cols per group
GW_E = EPG * ENEMY_F               # 256
PAIR_A = ALLY_F // 2               # 24
PAIR_E = ENEMY_F // 2              # 16
MA_FREE = NG * GW_A                # 12288
ME_FREE = NG * GW_E                # 8192
S2A_FREE = PAIR_A * E_C            # 6144
S2E_FREE = PAIR_E * E_C            # 4096
W2COLS = (PAIR_A + PAIR_E) * OUT   # 5120
KROWS = ALLY_F + ENEMY_F + 1       # 81 = stacked features + ones row
ELW = NG * ALLY_F + NG * ENEMY_F   # 2560 = fused el-block width (ally|enemy)
M_FREE = EPG * ELW                 # 20480 = fused M tile free dim
EOFF = NG * ALLY_F                 # 1536 = enemy col offset within el block

BATCH = 8                          # groups per prelu batch (8*128 = 1024 cols)
NBATCH = NG // BATCH               # 4
IMG_G = 8                          # leading groups delivered as dense host image
IMA_COLS = IMG_G * GW_A            # 3072
IME_COLS = IMG_G * GW_E            # 2048


def _ap(t, offset, dims):
    """Custom flat AP: dims = [(step, num), ...]; t is an AP or tensor handle."""
    a = t if isinstance(t, bass.AP) else t.ap()
    return dataclasses.replace(a, offset=offset, ap=[[s, n] for (s, n) in dims])


def build_program(alpha_a=0.25, alpha_e=0.25):
    assert alpha_a == alpha_e, "branches must share alpha"
    nc = bacc.Bacc("TRN2", target_bir_lowering=False, debug=False)

    fsp = nc.declare_dram_parameter("fsp", [KROWS, PROWS], BF16, isOutput=False)
    fx = nc.declare_dram_parameter("fx", [128, ELW], BF16, isOutput=False)
    w1e = nc.declare_dram_parameter("w1e", [KROWS, 128], BF16, isOutput=False)
    w2pack = nc.declare_dram_parameter("w2pack", [128, W2COLS], BF16, isOutput=False)
    out_d = nc.declare_dram_parameter("out", [OUT, E_C], F32, isOutput=True)

    with tile.TileContext(nc) as tc:
        _emit(nc, tc, fsp, fx, w1e, w2pack, out_d, alpha_a)
    nc.compile()
    return nc


def _emit(nc, tc, fsp, fx, w1e, w2pack, out_d, alpha):
    from contextlib import ExitStack

    Prelu = mybir.ActivationFunctionType.Prelu

    ctx = ExitStack()
    with ctx:
        const = ctx.enter_context(tc.tile_pool(name="const", bufs=1))
        work = ctx.enter_context(tc.tile_pool(name="work", bufs=1))
        psA = ctx.enter_context(tc.tile_pool(name="psA", bufs=3, space="PSUM"))
        psB = ctx.enter_context(tc.tile_pool(name="psB", bufs=3, space="PSUM"))
        psC = ctx.enter_context(tc.tile_pool(name="psC", bufs=1, space="PSUM"))
        psW = ctx.enter_context(tc.tile_pool(name="psW", bufs=1, space="PSUM"))

        # ---- persistent SBUF ----
        fsp_sb = const.tile([KROWS, PROWS], BF16)
        w1_sb = const.tile([KROWS, 128], BF16)
        w2_sb = const.tile([128, W2COLS], BF16)
        h_sb = work.tile([128, PROWS], BF16)
        m_sb = work.tile([128, M_FREE], BF16)
        s2a = work.tile([128, S2A_FREE], BF16)
        s2e = work.tile([128, S2E_FREE], BF16)
        osb = work.tile([OUT, E_C], F32)

        # ---- PE warmup: junk matmuls to flip the HAM clock gate to 8/8
        # before real work arrives (reads a memset junk tile) ----
        junk = work.tile([128, 64], BF16)
        nc.vector.memset(junk[:].bitcast(F32), 0.0)
        pw = psW.tile([128, 512], F32)
        for wi in range(56):
            nc.tensor.matmul(pw[0:64, 0:64], junk[:], junk[:],
                             start=True, stop=True)

        # ---- memsets for M off-diagonal zeros, 4 el-aligned pieces so
        # the el-DMAs can chase them (vector + gpsimd) ----
        m_f32 = m_sb[:].bitcast(F32)
        P4 = M_FREE // 8            # f32 cols per piece (2 el blocks)
        nc.vector.memset(m_f32[:, 0 * P4 : 1 * P4], 0.0)
        nc.gpsimd.memset(m_f32[:, 1 * P4 : 2 * P4], 0.0)
        nc.vector.memset(m_f32[:, 2 * P4 : 3 * P4], 0.0)
        nc.gpsimd.memset(m_f32[:, 3 * P4 : 4 * P4], 0.0)

        # ---- parameter loads ----
        # fsp chunks lead each HWDGE ring; w2 halves trail (big transfers
        # at ring tails so they don't clog the el-DMA completion slots).
        CHW = PROWS // 2
        nc.sync.dma_start(fsp_sb[:, 0:CHW], fsp.ap()[:, 0:CHW])
        nc.scalar.dma_start(w1_sb[:], w1e.ap())
        nc.scalar.dma_start(
            fsp_sb[:, CHW : 2 * CHW], fsp.ap()[:, CHW : 2 * CHW])

        # fused M diag el-DMAs: one per el, [16 partitions x 2560 cols]
        # contiguous from the compact host array (ally block | enemy block).
        for el in range(EPG):
            nc.sync.dma_start(
                _ap(m_sb, el * (PITCH * M_FREE + ELW), [
                    (M_FREE, PITCH), (1, ELW)]),
                fx.ap()[el * PITCH : (el + 1) * PITCH, :])

        nc.sync.dma_start(w2_sb[:, 0 : W2COLS // 2], w2pack.ap()[:, 0 : W2COLS // 2])
        nc.gpsimd.dma_start(w2_sb[:, W2COLS // 2 :], w2pack.ap()[:, W2COLS // 2 :])

        # ---- pipeline: per sub-batch of 4 groups: A matmuls + prelu;
        # per batch of 8 groups: B matmuls + evac casts ----
        copy_rr = [0]

        def evac(dst, src):
            if copy_rr[0] % 3 == 2:
                nc.scalar.copy(dst, src)
            else:
                nc.vector.tensor_copy(dst, src)
            copy_rr[0] += 1

        SUBG = 4                     # groups per psA tile / prelu

        def a_stage(sub):
            pa = psA.tile([128, SUBG * 128], F32, tag="psA")
            for j in range(SUBG):
                g = sub * SUBG + j
                nc.tensor.matmul(
                    pa[:, j * 128 : (j + 1) * 128],
                    fsp_sb[:, g * 128 : (g + 1) * 128],
                    w1_sb[:],
                    start=True, stop=True)
            nc.scalar.activation(
                h_sb[:, sub * 512 : (sub + 1) * 512], pa[:],
                Prelu, scale=1.0, alpha=alpha)

        def b_ally(g0):              # 2 groups per PSUM tile
            pb = psB.tile([128, 512], F32, tag="psB")
            for u in range(2):
                g = g0 + u
                lhsT = h_sb[:, g * 128 : g * 128 + 64]
                nc.tensor.matmul(
                    pb[64:128, u * 192 : (u + 1) * 192], lhsT,
                    _ap(m_sb, g * ALLY_F + PAIR_A,
                        [(M_FREE, 128), (ELW, EPG), (1, PAIR_A)]),
                    start=True, stop=True)
                nc.tensor.matmul(
                    pb[0:64, u * 192 : (u + 1) * 192], lhsT,
                    _ap(m_sb, g * ALLY_F,
                        [(M_FREE, 128), (ELW, EPG), (1, PAIR_A)]),
                    start=True, stop=True)
            evac(
                _ap(s2a, g0 * EPG,
                    [(S2A_FREE, 128), (E_C, PAIR_A), (1, 2 * EPG)]),
                _ap(pb, 0,
                    [(512, 128), (1, PAIR_A), (192, 2), (PAIR_A, EPG)]))

        def b_enemy(g0):             # 4 groups per PSUM tile
            pb = psB.tile([128, 512], F32, tag="psB")
            for u in range(4):
                g = g0 + u
                lhsT = h_sb[:, g * 128 + 64 : g * 128 + 128]
                nc.tensor.matmul(
                    pb[64:128, u * 128 : (u + 1) * 128], lhsT,
                    _ap(m_sb, EOFF + g * ENEMY_F + PAIR_E,
                        [(M_FREE, 128), (ELW, EPG), (1, PAIR_E)]),
                    start=True, stop=True)
                nc.tensor.matmul(
                    pb[0:64, u * 128 : (u + 1) * 128], lhsT,
                    _ap(m_sb, EOFF + g * ENEMY_F,
                        [(M_FREE, 128), (ELW, EPG), (1, PAIR_E)]),
                    start=True, stop=True)
            evac(
                _ap(s2e, g0 * EPG,
                    [(S2E_FREE, 128), (E_C, PAIR_E), (1, 4 * EPG)]),
                _ap(pb, 0,
                    [(512, 128), (1, PAIR_E), (128, 4), (PAIR_E, EPG)]))

        for sub in range(NG // SUBG):
            a_stage(sub)
        for g0 in range(0, NG, 2):
            b_ally(g0)
        for g0 in range(0, NG, 4):
            b_enemy(g0)

        # ---- stage C: out_T[o,e] accumulation over 40 fp slices ----
        pc = psC.tile([OUT, E_C], F32)
        n_sl = PAIR_A + PAIR_E
        idx = 0
        for fp in range(PAIR_A):
            nc.tensor.matmul(
                pc[:], w2_sb[:, fp * OUT : (fp + 1) * OUT],
                s2a[:, fp * E_C : (fp + 1) * E_C],
                start=(idx == 0), stop=(idx == n_sl - 1))
            idx += 1
        for fp in range(PAIR_E):
            nc.tensor.matmul(
                pc[:], w2_sb[:, (PAIR_A + fp) * OUT : (PAIR_A + fp + 1) * OUT],
                s2e[:, fp * E_C : (fp + 1) * E_C],
                start=(idx == 0), stop=(idx == n_sl - 1))
            idx += 1

        nc.vector.tensor_copy(osb[:], pc[:])
        nc.sync.dma_start(out_d.ap(), osb[:])


@functools.lru_cache(maxsize=2)
def _cached_program(alpha_a, alpha_e):
    return build_program(alpha_a, alpha_e)


def host_prep(ally_features, enemy_features, Wa1, ba1, aa, Wa2, ba2,
              We1, be1, ae, We2, be2):
    """Per-core input maps (numpy, bf16) + host-side bias term."""
    bf = ml_dtypes.bfloat16

    def uniform_alpha(a):
        a = np.asarray(a, dtype=np.float32)
        assert np.allclose(a, a[0]), "per-channel alpha not supported"
        return float(a[0])

    ua, ue = uniform_alpha(aa), uniform_alpha(ae)

    w1 = np.zeros((KROWS, 128), dtype=np.float32)
    w1[0:ALLY_F, 0:HYPER] = np.asarray(Wa1)
    w1[ALLY_F:80, HYPER:128] = np.asarray(We1)
    w1[80, 0:HYPER] = np.asarray(ba1)
    w1[80, HYPER:128] = np.asarray(be1)
    w1 = w1.astype(bf)

    w2 = np.zeros((128, W2COLS), dtype=np.float32)
    Wa2_, We2_ = np.asarray(Wa2), np.asarray(We2)
    for fp in range(PAIR_A):
        w2[0:HYPER, fp * OUT : (fp + 1) * OUT] = Wa2_[:, fp * OUT : (fp + 1) * OUT]
        w2[HYPER:128, fp * OUT : (fp + 1) * OUT] = \
            Wa2_[:, (fp + PAIR_A) * OUT : (fp + PAIR_A + 1) * OUT]
    for fp in range(PAIR_E):
        c = (PAIR_A + fp) * OUT
        w2[0:HYPER, c : c + OUT] = We2_[:, fp * OUT : (fp + 1) * OUT]
        w2[HYPER:128, c : c + OUT] = \
            We2_[:, (fp + PAIR_E) * OUT : (fp + PAIR_E + 1) * OUT]
    w2 = w2.astype(bf)

    fa_all = np.asarray(ally_features, dtype=np.float32)
    fe_all = np.asarray(enemy_features, dtype=np.float32)
    fa_bf = fa_all.astype(bf)
    fe_bf = fe_all.astype(bf)

    # host-side bias term: fsum @ b2 (exact fp32)
    fsum_a = fa_all.reshape(B_FULL, N_AGENTS, ALLY_F).sum(axis=1)
    fsum_e = fe_all.reshape(B_FULL, N_ENEMIES, ENEMY_F).sum(axis=1)
    bias_out = (fsum_a @ np.asarray(ba2).reshape(ALLY_F, OUT)
                + fsum_e @ np.asarray(be2).reshape(ENEMY_F, OUT)).astype(np.float32)

    RA = E_C * N_AGENTS
    RE = E_C * N_ENEMIES
    in_maps = []
    for c in range(N_CORES):
        fa_c = fa_bf[c * RA : (c + 1) * RA].reshape(E_C, N_AGENTS, ALLY_F)
        fe_c = fe_bf[c * RE : (c + 1) * RE].reshape(E_C, N_ENEMIES, ENEMY_F)
        # padded row-major features [E_C*16, featf], pad rows zero
        fa_pad = np.zeros((E_C, PITCH, ALLY_F), dtype=bf)
        fa_pad[:, :N_AGENTS, :] = fa_c
        fe_pad = np.zeros((E_C, PITCH, ENEMY_F), dtype=bf)
        fe_pad[:, :N_ENEMIES, :] = fe_c
        fa_pad = fa_pad.reshape(PROWS, ALLY_F)
        fe_pad = fe_pad.reshape(PROWS, ENEMY_F)
        # fsp: [81, PROWS] = [ally F^T padded; enemy F^T padded; ones]
        fs = np.zeros((KROWS, PROWS), dtype=bf)
        fs[0:ALLY_F, :] = fa_pad.T
        fs[ALLY_F:80, :] = fe_pad.T
        fs[80, :] = np.float32(1.0)

        # compact el-major diag arrays: row p=(el*16+i), col g*featf + f
        def m_compact(f_pad, featf):
            f4 = f_pad.reshape(NG, EPG, PITCH, featf)
            return np.ascontiguousarray(
                f4.transpose(1, 2, 0, 3).reshape(128, NG * featf))

        in_maps.append({
            "fsp": np.ascontiguousarray(fs),
            "fx": np.ascontiguousarray(np.concatenate(
                [m_compact(fa_pad, ALLY_F), m_compact(fe_pad, ENEMY_F)],
                axis=1)),
            "w1e": w1, "w2pack": w2,
        })
    aux = {"bias_out": bias_out, "ua": ua, "ue": ue}
    return in_maps, aux


def assemble_output(results, aux):
    outs = [np.asarray(r["out"], dtype=np.float32) for r in results]
    dev = np.concatenate([o.T for o in outs], axis=0)
    return dev + aux["bias_out"]


def kernel(**inputs) -> np.ndarray:
    in_maps, aux = host_prep(**inputs)
    nc = _cached_program(aux["ua"], aux["ue"])
    res = run_bass_kernel_spmd(nc, in_maps, core_ids=list(range(N_CORES)))
    return assemble_output(res.results, aux)


if __name__ == "__main__":
    build_program()
    print("built ok")
